# revision 14
# baseline (speedup 1.0000x reference)
"""Batch-Jacobian of a 3-layer tanh MLP (64->256->256->64), B=8192.

J[b] = W3^T diag(1-h2^2) W2^T diag(1-h1^2) W1^T   (shape 64x64 per b)

v2 strategy (per core, 1024 batch elems, 4 chunks of 256):
  host precomputes G[i][k,m] = W1[i,k]*W2[k,m]  (bf16, DMA'd in)
  forward (feature layout) -> d1[k,b], d2[m,b]
  stage1 (PE): V[m,(i,b)] = sum_k G_i[k,m] d1[k,b]   (moving = d1, no DVE build)
  drain: w = V * d2  split DVE-direct / ACT-copy + DVE-mult(2x bf16)
  stage2 (PE): J[j,(b,i)] = sum_m W3[m,j] w[m,i,b], two 64-row outputs
               packed into one 128-partition psum tile (base_partition 0/64)
  output: DMA psum -> DRAM directly (256B contiguous runs), no drain
"""

import sys

sys.path.insert(0, "/opt/trn_rl_repo")

import numpy as np
import ml_dtypes
from contextlib import ExitStack

import concourse.bass as bass
import concourse.mybir as mybir
import concourse.tile as tile
from concourse import bacc
from concourse.bass_utils import run_bass_kernel_spmd

B, D, H = 8192, 64, 256
NCORES = 8
BS = B // NCORES  # 1024 batch per core
CHUNK = 256  # jacobian batch chunk
NCH = BS // CHUNK  # 4
NQ = 16  # i-quads (4 i per quad)
WIN = 16  # batch elems per output DMA (2 halves of 8)

BF = mybir.dt.bfloat16
F32 = mybir.dt.float32
MUL = mybir.AluOpType.mult
ADD = mybir.AluOpType.add
Tanh = mybir.ActivationFunctionType.Tanh
Copy = mybir.ActivationFunctionType.Copy

_CACHE = {}
TRACE = False
# fraction pattern for DVE-direct drains: 1 of every 5 tiles
DVE_DIRECT_EVERY = 5


def _build():
    nc = bacc.Bacc("TRN2")
    xT_d = nc.dram_tensor("xt", [D, BS], BF, kind="ExternalInput")
    g_d = nc.dram_tensor("g", [2, 128, D * H], BF, kind="ExternalInput")
    w1_d = nc.dram_tensor("w1", [D, H], BF, kind="ExternalInput")
    w2_d = nc.dram_tensor("w2", [2, 128, H], BF, kind="ExternalInput")
    w3_d = nc.dram_tensor("w3", [2, 128, D], BF, kind="ExternalInput")
    b1_d = nc.dram_tensor("b1", [H], F32, kind="ExternalInput")
    b2_d = nc.dram_tensor("b2", [H], F32, kind="ExternalInput")
    jac_d = nc.dram_tensor("jac", [BS, D, D], F32, kind="ExternalOutput")

    with ExitStack() as ctx:
        tc = ctx.enter_context(tile.TileContext(nc))
        const = ctx.enter_context(tc.tile_pool(name="const", bufs=1))
        sb = ctx.enter_context(tc.tile_pool(name="sb", bufs=2))
        ps = ctx.enter_context(tc.tile_pool(name="ps", bufs=1, space="PSUM"))

        # ---- constants ----
        xT_sb = const.tile([D, BS], BF)
        nc.sync.dma_start(out=xT_sb, in_=xT_d[:, :])
        g_sb = [const.tile([128, D * H], BF, name=f"g{k}") for k in range(2)]
        w2_sb = [const.tile([128, H], BF, name=f"w2{k}") for k in range(2)]
        w3_sb = [const.tile([128, D], BF, name=f"w3{k}") for k in range(2)]
        for k in range(2):
            # split the big G load into 8-i-group pieces so stage 1 can start
            # as soon as the first groups land
            for gq in range(8):
                sl = slice(gq * 8 * H, (gq + 1) * 8 * H)
                nc.sync.dma_start(out=g_sb[k][:, sl], in_=g_d[k][:, sl])
            nc.sync.dma_start(out=w2_sb[k], in_=w2_d[k])
            nc.sync.dma_start(out=w3_sb[k], in_=w3_d[k])
        w1_sb = const.tile([D, H], BF)
        nc.sync.dma_start(out=w1_sb, in_=w1_d[:, :])
        b1_sb = const.tile([128, 2], F32)
        b2_sb = const.tile([128, 2], F32)
        nc.sync.dma_start(out=b1_sb, in_=b1_d.rearrange("(a p) -> p a", p=128))
        nc.sync.dma_start(out=b2_sb, in_=b2_d.rearrange("(a p) -> p a", p=128))

        # ---- forward: d1[k,b], d2[m,b] for all 1024 b ----
        h1 = [const.tile([128, BS], BF, name=f"h1_{k}") for k in range(2)]
        d1 = [const.tile([128, BS], BF, name=f"d1_{k}") for k in range(2)]
        d2 = [const.tile([128, BS], BF, name=f"d2_{m}") for m in range(2)]
        for hh in range(2):
            a_ps = ps.tile([128, BS], F32, tag="v", bufs=3, name="a1_ps")
            for s in range(BS // 512):
                nc.tensor.matmul(
                    a_ps[:, s * 512 : (s + 1) * 512],
                    w1_sb[:, hh * 128 : (hh + 1) * 128],
                    xT_sb[:, s * 512 : (s + 1) * 512],
                    start=True,
                    stop=True,
                )
            nc.scalar.activation(
                out=h1[hh], in_=a_ps, func=Tanh, bias=b1_sb[:, hh : hh + 1]
            )
            sq = sb.tile([128, BS], BF, tag="sq", name="sq1")
            nc.vector.tensor_tensor(out=sq, in0=h1[hh], in1=h1[hh], op=MUL)
            nc.vector.tensor_scalar(
                out=d1[hh], in0=sq, scalar1=-1.0, scalar2=1.0, op0=MUL, op1=ADD
            )
        for mh in range(2):
            a_ps = ps.tile([128, BS], F32, tag="v", bufs=3, name="a2_ps")
            for s in range(BS // 512):
                for hh in range(2):
                    nc.tensor.matmul(
                        a_ps[:, s * 512 : (s + 1) * 512],
                        w2_sb[hh][:, mh * 128 : (mh + 1) * 128],
                        h1[hh][:, s * 512 : (s + 1) * 512],
                        start=(hh == 0),
                        stop=(hh == 1),
                    )
            h2 = sb.tile([128, BS], BF, tag="h2", name="h2")
            nc.scalar.activation(
                out=h2, in_=a_ps, func=Tanh, bias=b2_sb[:, mh : mh + 1]
            )
            sq = sb.tile([128, BS], BF, tag="sq", name="sq2")
            nc.vector.tensor_tensor(out=sq, in0=h2, in1=h2, op=MUL)
            nc.vector.tensor_scalar(
                out=d2[mh], in0=sq, scalar1=-1.0, scalar2=1.0, op0=MUL, op1=ADD
            )

        # single w buffer per mh, WINDOW-major I-major: w[p, t*512 + i*8 + bl]
        # (t = 8-batch window). Stage-2 moving is a contiguous 512-col slice;
        # stage-1 drain writes land in contiguous 16B runs (8 bl x bf16).
        w_sb = [const.tile([128, CHUNK * D], BF, name=f"w_{m}") for m in range(2)]

        tidx = 0
        for c in range(NCH):
            cb = c * CHUNK
            # ---- stage 1: V[m,(i,b)] = sum_k G_i[k,m] d1[k,b]; w = V*d2 ----
            for q in range(NQ):
                for mh in range(2):
                    v_ps = ps.tile([128, 4 * CHUNK], F32, tag="v", bufs=3, name="v_ps")
                    for qi in range(4):
                        i = q * 4 + qi
                        goff = i * H + mh * 128
                        for kh in range(2):
                            nc.tensor.matmul(
                                v_ps[:, qi * CHUNK : (qi + 1) * CHUNK],
                                g_sb[kh][:, goff : goff + 128],
                                d1[kh][:, cb : cb + CHUNK],
                                start=(kh == 0),
                                stop=(kh == 1),
                            )
                    # drain: w[p, t*512 + i*8 + bl] = V * d2
                    # out iterates (qi, t, bl): innermost bl is 8-elem
                    # contiguous (one 16B line) - near-contiguous writes
                    wout = (
                        w_sb[mh]
                        .rearrange("p (t i bl) -> p i t bl", t=32, i=D)
                        [:, q * 4 : q * 4 + 4]
                    )
                    d2b = (
                        d2[mh][:, None, cb : cb + CHUNK]
                        .broadcast_to([128, 4, CHUNK])
                        .rearrange("p i (t bl) -> p i t bl", t=32)
                    )
                    vv = v_ps.rearrange("p (i t bl) -> p i t bl", i=4, t=32)
                    # GP-path tiles go EARLY in the chunk (their copy+mult
                    # chain has the longest latency; the chunk barrier cares
                    # about the last drain) - DVE handles the tail
                    if q * 2 + mh < 10:
                        # ACT copies psum out; GPSIMD does the multiply
                        vtmp = sb.tile([128, 4 * CHUNK], BF, tag="vtmp", bufs=3,
                                       name="vtmp")
                        nc.scalar.activation(out=vtmp, in_=v_ps, func=Copy)
                        nc.gpsimd.tensor_tensor(
                            out=wout,
                            in0=vtmp.rearrange("p (i t bl) -> p i t bl", i=4, t=32),
                            in1=d2b,
                            op=MUL,
                        )
                    else:
                        nc.vector.tensor_tensor(out=wout, in0=vv, in1=d2b, op=MUL)
                    tidx += 1

            # ---- stage 2: J[j,(b,i)]; two 64-row halves packed on partitions ----
            for t in range(CHUNK // WIN):
                j_ps = ps.tile([128, 8 * D], F32, tag="js", bufs=2, name="j_ps")
                for half in range(2):
                    bo = t * WIN + half * 8
                    for mh in range(2):
                        nc.tensor.matmul(
                            j_ps[half * 64 : (half + 1) * 64, :],
                            w3_sb[mh],
                            w_sb[mh][:, bo * D : (bo + 8) * D],
                            start=(mh == 0),
                            stop=(mh == 1),
                        )
                # psum cols are (i*8+bl) i-major; permute to (bl*64+i) for the
                # DMA here (ACT strided write, quarter the stage-1 volume)
                jbuf = sb.tile([128, 8 * D], F32, tag="jbuf", bufs=3, name="jbuf")
                nc.scalar.activation(
                    out=jbuf.rearrange("p (bl i) -> p i bl", bl=8),
                    in_=j_ps.rearrange("p (i bl) -> p i bl", i=D),
                    func=Copy,
                )
                b0 = cb + t * WIN
                for half in range(2):
                    nc.sync.dma_start(
                        out=jac_d[b0 + half * 8 : b0 + half * 8 + 8].rearrange(
                            "bl j i -> j bl i"
                        ),
                        in_=jbuf[half * 64 : (half + 1) * 64, :].rearrange(
                            "p (bl i) -> p bl i", bl=8
                        ),
                    )
    nc.compile()
    return nc


def kernel(x, W1, b1, W2, b2, W3, b3):
    x = np.asarray(x, dtype=np.float32)
    bf = ml_dtypes.bfloat16
    if "nc" not in _CACHE:
        _CACHE["nc"] = _build()
    nc = _CACHE["nc"]

    W1f = np.asarray(W1, np.float32)
    W2f = np.asarray(W2, np.float32)
    W3f = np.asarray(W3, np.float32)
    # G[kh][k', i*256 + m] = W1[i, kh*128+k'] * W2[kh*128+k', m]
    w1t = np.ascontiguousarray(W1f.T).reshape(2, 128, D)  # (kh, k', i)
    w2r = W2f.reshape(2, 128, H)  # (kh, k', m)
    g = (w1t[:, :, :, None] * w2r[:, :, None, :]).reshape(2, 128, D * H)

    shared = {
        "g": g.astype(bf),
        "w1": W1f.astype(bf),
        "w2": w2r.astype(bf),
        "w3": np.ascontiguousarray(W3f.reshape(2, 128, D)).astype(bf),
        "b1": np.asarray(b1, np.float32),
        "b2": np.asarray(b2, np.float32),
    }
    in_maps = [
        {
            "xt": np.ascontiguousarray(x[c * BS : (c + 1) * BS].T).astype(bf),
            **shared,
        }
        for c in range(NCORES)
    ]
    res = run_bass_kernel_spmd(
        nc, in_maps, core_ids=list(range(NCORES)), trace=TRACE
    )
    _CACHE["last_res"] = res
    return np.concatenate([r["jac"] for r in res.results], axis=0)


# revision 16
# speedup vs baseline: 1.1847x; 1.1847x over previous
"""Batch-Jacobian of a 3-layer tanh MLP (64->256->256->64), B=8192.

J[b] = W3^T diag(1-h2^2) W2^T diag(1-h1^2) W1^T   (shape 64x64 per b)

v2 strategy (per core, 1024 batch elems, 4 chunks of 256):
  host precomputes G[i][k,m] = W1[i,k]*W2[k,m]  (bf16, DMA'd in)
  forward (feature layout) -> d1[k,b], d2[m,b]
  stage1 (PE): V[m,(i,b)] = sum_k G_i[k,m] d1[k,b]   (moving = d1, no DVE build)
  drain: w = V * d2  split DVE-direct / ACT-copy + DVE-mult(2x bf16)
  stage2 (PE): J[j,(b,i)] = sum_m W3[m,j] w[m,i,b], two 64-row outputs
               packed into one 128-partition psum tile (base_partition 0/64)
  output: DMA psum -> DRAM directly (256B contiguous runs), no drain
"""

import sys

sys.path.insert(0, "/opt/trn_rl_repo")

import numpy as np
import ml_dtypes
from contextlib import ExitStack

import concourse.bass as bass
import concourse.mybir as mybir
import concourse.tile as tile
from concourse import bacc
from concourse.bass_utils import run_bass_kernel_spmd

B, D, H = 8192, 64, 256
NCORES = 8
BS = B // NCORES  # 1024 batch per core
CHUNK = 256  # jacobian batch chunk
NCH = BS // CHUNK  # 4
NQ = 16  # i-quads (4 i per quad)
WIN = 16  # batch elems per output DMA (2 halves of 8)

BF = mybir.dt.bfloat16
F32 = mybir.dt.float32
MUL = mybir.AluOpType.mult
ADD = mybir.AluOpType.add
Tanh = mybir.ActivationFunctionType.Tanh
Copy = mybir.ActivationFunctionType.Copy

_CACHE = {}
TRACE = False
# fraction pattern for DVE-direct drains: 1 of every 5 tiles
DVE_DIRECT_EVERY = 5


def _build():
    nc = bacc.Bacc("TRN2")
    xT_d = nc.dram_tensor("xt", [D, BS], BF, kind="ExternalInput")
    g_d = nc.dram_tensor("g", [2, 128, D * H], BF, kind="ExternalInput")
    w1_d = nc.dram_tensor("w1", [D, H], BF, kind="ExternalInput")
    w2_d = nc.dram_tensor("w2", [2, 128, H], BF, kind="ExternalInput")
    w3_d = nc.dram_tensor("w3", [2, 128, D], BF, kind="ExternalInput")
    b1_d = nc.dram_tensor("b1", [H], F32, kind="ExternalInput")
    b2_d = nc.dram_tensor("b2", [H], F32, kind="ExternalInput")
    jac_d = nc.dram_tensor("jac", [BS, D, D], F32, kind="ExternalOutput")

    with ExitStack() as ctx:
        tc = ctx.enter_context(tile.TileContext(nc))
        const = ctx.enter_context(tc.tile_pool(name="const", bufs=1))
        sb = ctx.enter_context(tc.tile_pool(name="sb", bufs=2))
        ps = ctx.enter_context(tc.tile_pool(name="ps", bufs=1, space="PSUM"))

        # ---- constants ----
        xT_sb = const.tile([D, BS], BF)
        nc.sync.dma_start(out=xT_sb, in_=xT_d[:, :])
        g_sb = [const.tile([128, D * H], BF, name=f"g{k}") for k in range(2)]
        w2_sb = [const.tile([128, H], BF, name=f"w2{k}") for k in range(2)]
        w3_sb = [const.tile([128, D], BF, name=f"w3{k}") for k in range(2)]
        for k in range(2):
            # split the big G load into 8-i-group pieces so stage 1 can start
            # as soon as the first groups land
            for gq in range(8):
                sl = slice(gq * 8 * H, (gq + 1) * 8 * H)
                nc.sync.dma_start(out=g_sb[k][:, sl], in_=g_d[k][:, sl])
            nc.sync.dma_start(out=w2_sb[k], in_=w2_d[k])
            nc.sync.dma_start(out=w3_sb[k], in_=w3_d[k])
        w1_sb = const.tile([D, H], BF)
        nc.sync.dma_start(out=w1_sb, in_=w1_d[:, :])
        b1_sb = const.tile([128, 2], F32)
        b2_sb = const.tile([128, 2], F32)
        nc.sync.dma_start(out=b1_sb, in_=b1_d.rearrange("(a p) -> p a", p=128))
        nc.sync.dma_start(out=b2_sb, in_=b2_d.rearrange("(a p) -> p a", p=128))

        # ---- forward: d1[k,b], d2[m,b] for all 1024 b ----
        h1 = [const.tile([128, BS], BF, name=f"h1_{k}") for k in range(2)]
        d1 = [const.tile([128, BS], BF, name=f"d1_{k}") for k in range(2)]
        d2 = [const.tile([128, BS], BF, name=f"d2_{m}") for m in range(2)]
        for hh in range(2):
            a_ps = ps.tile([128, BS], F32, tag="v", bufs=3, name="a1_ps")
            for s in range(BS // 512):
                nc.tensor.matmul(
                    a_ps[:, s * 512 : (s + 1) * 512],
                    w1_sb[:, hh * 128 : (hh + 1) * 128],
                    xT_sb[:, s * 512 : (s + 1) * 512],
                    start=True,
                    stop=True,
                )
            nc.scalar.activation(
                out=h1[hh], in_=a_ps, func=Tanh, bias=b1_sb[:, hh : hh + 1]
            )
            sq = sb.tile([128, BS], BF, tag="sq", name="sq1")
            nc.vector.tensor_tensor(out=sq, in0=h1[hh], in1=h1[hh], op=MUL)
            nc.vector.tensor_scalar(
                out=d1[hh], in0=sq, scalar1=-1.0, scalar2=1.0, op0=MUL, op1=ADD
            )
        for mh in range(2):
            a_ps = ps.tile([128, BS], F32, tag="v", bufs=3, name="a2_ps")
            for s in range(BS // 512):
                for hh in range(2):
                    nc.tensor.matmul(
                        a_ps[:, s * 512 : (s + 1) * 512],
                        w2_sb[hh][:, mh * 128 : (mh + 1) * 128],
                        h1[hh][:, s * 512 : (s + 1) * 512],
                        start=(hh == 0),
                        stop=(hh == 1),
                    )
            h2 = sb.tile([128, BS], BF, tag="h2", name="h2")
            nc.scalar.activation(
                out=h2, in_=a_ps, func=Tanh, bias=b2_sb[:, mh : mh + 1]
            )
            sq = sb.tile([128, BS], BF, tag="sq", name="sq2")
            nc.vector.tensor_tensor(out=sq, in0=h2, in1=h2, op=MUL)
            nc.vector.tensor_scalar(
                out=d2[mh], in0=sq, scalar1=-1.0, scalar2=1.0, op0=MUL, op1=ADD
            )

        # single w buffer per mh, WINDOW-major I-major: w[p, t*512 + i*8 + bl]
        # (t = 8-batch window). Stage-2 moving is a contiguous 512-col slice;
        # stage-1 drain writes land in contiguous 16B runs (8 bl x bf16).
        w_sb = [const.tile([128, CHUNK * D], BF, name=f"w_{m}") for m in range(2)]

        tidx = 0
        for c in range(NCH):
            cb = c * CHUNK
            # ---- stage 1: V[m,(i,b)] = sum_k G_i[k,m] d1[k,b]; w = V*d2 ----
            for q in range(NQ):
                for mh in range(2):
                    v_ps = ps.tile([128, 4 * CHUNK], F32, tag="v", bufs=3, name="v_ps")
                    for qi in range(4):
                        i = q * 4 + qi
                        goff = i * H + mh * 128
                        for kh in range(2):
                            nc.tensor.matmul(
                                v_ps[:, qi * CHUNK : (qi + 1) * CHUNK],
                                g_sb[kh][:, goff : goff + 128],
                                d1[kh][:, cb : cb + CHUNK],
                                start=(kh == 0),
                                stop=(kh == 1),
                            )
                    # drain: w[p, t*512 + i*8 + bl] = V * d2
                    # out iterates (qi, t, bl): innermost bl is 8-elem
                    # contiguous (one 16B line) - near-contiguous writes
                    wout = (
                        w_sb[mh]
                        .rearrange("p (t i bl) -> p i t bl", t=32, i=D)
                        [:, q * 4 : q * 4 + 4]
                    )
                    d2b = (
                        d2[mh][:, None, cb : cb + CHUNK]
                        .broadcast_to([128, 4, CHUNK])
                        .rearrange("p i (t bl) -> p i t bl", t=32)
                    )
                    vv = v_ps.rearrange("p (i t bl) -> p i t bl", i=4, t=32)
                    # every 3rd tile takes the ACT-copy + GPSIMD-multiply path
                    # so DVE keeps pace with the PE tile rate
                    if (q * 2 + mh) % 3 == 0:
                        # ACT copies psum out; GPSIMD does the multiply
                        vtmp = sb.tile([128, 4 * CHUNK], BF, tag="vtmp", bufs=3,
                                       name="vtmp")
                        nc.scalar.activation(out=vtmp, in_=v_ps, func=Copy)
                        nc.gpsimd.tensor_tensor(
                            out=wout,
                            in0=vtmp.rearrange("p (i t bl) -> p i t bl", i=4, t=32),
                            in1=d2b,
                            op=MUL,
                        )
                    else:
                        nc.vector.tensor_tensor(out=wout, in0=vv, in1=d2b, op=MUL)
                    tidx += 1

            # ---- stage 2: J[j,(b,i)]; two 64-row halves packed on partitions ----
            for t in range(CHUNK // WIN):
                j_ps = ps.tile([128, 8 * D], F32, tag="js", bufs=2, name="j_ps")
                for half in range(2):
                    bo = t * WIN + half * 8
                    for mh in range(2):
                        nc.tensor.matmul(
                            j_ps[half * 64 : (half + 1) * 64, :],
                            w3_sb[mh],
                            w_sb[mh][:, bo * D : (bo + 8) * D],
                            start=(mh == 0),
                            stop=(mh == 1),
                        )
                # psum cols are (i*8+bl) i-major; permute to (bl*64+i) for the
                # DMA here. Iterate (bl, i): strided psum READ, contiguous
                # jbuf WRITE (strided writes are ~4x slow; reads are cheap)
                jbuf = sb.tile([128, 8 * D], F32, tag="jbuf", bufs=3, name="jbuf")
                nc.scalar.activation(
                    out=jbuf,
                    in_=j_ps.rearrange("p (i bl) -> p bl i", i=D),
                    func=Copy,
                )
                b0 = cb + t * WIN
                for half in range(2):
                    nc.sync.dma_start(
                        out=jac_d[b0 + half * 8 : b0 + half * 8 + 8].rearrange(
                            "bl j i -> j bl i"
                        ),
                        in_=jbuf[half * 64 : (half + 1) * 64, :].rearrange(
                            "p (bl i) -> p bl i", bl=8
                        ),
                    )
    nc.compile()
    return nc


def kernel(x, W1, b1, W2, b2, W3, b3):
    x = np.asarray(x, dtype=np.float32)
    bf = ml_dtypes.bfloat16
    if "nc" not in _CACHE:
        _CACHE["nc"] = _build()
    nc = _CACHE["nc"]

    W1f = np.asarray(W1, np.float32)
    W2f = np.asarray(W2, np.float32)
    W3f = np.asarray(W3, np.float32)
    # G[kh][k', i*256 + m] = W1[i, kh*128+k'] * W2[kh*128+k', m]
    w1t = np.ascontiguousarray(W1f.T).reshape(2, 128, D)  # (kh, k', i)
    w2r = W2f.reshape(2, 128, H)  # (kh, k', m)
    g = (w1t[:, :, :, None] * w2r[:, :, None, :]).reshape(2, 128, D * H)

    shared = {
        "g": g.astype(bf),
        "w1": W1f.astype(bf),
        "w2": w2r.astype(bf),
        "w3": np.ascontiguousarray(W3f.reshape(2, 128, D)).astype(bf),
        "b1": np.asarray(b1, np.float32),
        "b2": np.asarray(b2, np.float32),
    }
    in_maps = [
        {
            "xt": np.ascontiguousarray(x[c * BS : (c + 1) * BS].T).astype(bf),
            **shared,
        }
        for c in range(NCORES)
    ]
    res = run_bass_kernel_spmd(
        nc, in_maps, core_ids=list(range(NCORES)), trace=TRACE
    )
    _CACHE["last_res"] = res
    return np.concatenate([r["jac"] for r in res.results], axis=0)


# revision 18
# speedup vs baseline: 1.2602x; 1.0637x over previous
"""Batch-Jacobian of a 3-layer tanh MLP (64->256->256->64), B=8192.

J[b] = W3^T diag(1-h2^2) W2^T diag(1-h1^2) W1^T   (shape 64x64 per b)

v2 strategy (per core, 1024 batch elems, 4 chunks of 256):
  host precomputes G[i][k,m] = W1[i,k]*W2[k,m]  (bf16, DMA'd in)
  forward (feature layout) -> d1[k,b], d2[m,b]
  stage1 (PE): V[m,(i,b)] = sum_k G_i[k,m] d1[k,b]   (moving = d1, no DVE build)
  drain: w = V * d2  split DVE-direct / ACT-copy + DVE-mult(2x bf16)
  stage2 (PE): J[j,(b,i)] = sum_m W3[m,j] w[m,i,b], two 64-row outputs
               packed into one 128-partition psum tile (base_partition 0/64)
  output: DMA psum -> DRAM directly (256B contiguous runs), no drain
"""

import sys

sys.path.insert(0, "/opt/trn_rl_repo")

import numpy as np
import ml_dtypes
from contextlib import ExitStack

import concourse.bass as bass
import concourse.mybir as mybir
import concourse.tile as tile
from concourse import bacc
from concourse.bass_utils import run_bass_kernel_spmd

B, D, H = 8192, 64, 256
NCORES = 8
BS = B // NCORES  # 1024 batch per core
CHUNK = 256  # jacobian batch chunk
NCH = BS // CHUNK  # 4
NQ = 16  # i-quads (4 i per quad)
WIN = 16  # batch elems per output DMA (2 halves of 8)

BF = mybir.dt.bfloat16
F32 = mybir.dt.float32
MUL = mybir.AluOpType.mult
ADD = mybir.AluOpType.add
Tanh = mybir.ActivationFunctionType.Tanh
Copy = mybir.ActivationFunctionType.Copy

_CACHE = {}
TRACE = False
# fraction pattern for DVE-direct drains: 1 of every 5 tiles
DVE_DIRECT_EVERY = 5


def _build():
    nc = bacc.Bacc("TRN2")
    xT_d = nc.dram_tensor("xt", [D, BS], BF, kind="ExternalInput")
    g_d = nc.dram_tensor("g", [2, 128, D * H], BF, kind="ExternalInput")
    w1_d = nc.dram_tensor("w1", [D, H], BF, kind="ExternalInput")
    w2_d = nc.dram_tensor("w2", [2, 128, H], BF, kind="ExternalInput")
    w3_d = nc.dram_tensor("w3", [2, 128, D], BF, kind="ExternalInput")
    b1_d = nc.dram_tensor("b1", [H], F32, kind="ExternalInput")
    b2_d = nc.dram_tensor("b2", [H], F32, kind="ExternalInput")
    jac_d = nc.dram_tensor("jac", [BS, D, D], F32, kind="ExternalOutput")

    with ExitStack() as ctx:
        tc = ctx.enter_context(tile.TileContext(nc))
        const = ctx.enter_context(tc.tile_pool(name="const", bufs=1))
        sb = ctx.enter_context(tc.tile_pool(name="sb", bufs=2))
        ps = ctx.enter_context(tc.tile_pool(name="ps", bufs=1, space="PSUM"))

        # ---- constants: small tensors first (forward can start right away);
        # G streams on the scalar HWDGE queue, kh-interleaved so stage-1's
        # first i-groups are ready early ----
        xT_sb = const.tile([D, BS], BF)
        nc.sync.dma_start(out=xT_sb, in_=xT_d[:, :])
        g_sb = [const.tile([128, D * H], BF, name=f"g{k}") for k in range(2)]
        w2_sb = [const.tile([128, H], BF, name=f"w2{k}") for k in range(2)]
        w3_sb = [const.tile([128, D], BF, name=f"w3{k}") for k in range(2)]
        w1_sb = const.tile([D, H], BF)
        nc.sync.dma_start(out=w1_sb, in_=w1_d[:, :])
        b1_sb = const.tile([128, 2], F32)
        b2_sb = const.tile([128, 2], F32)
        nc.sync.dma_start(out=b1_sb, in_=b1_d.rearrange("(a p) -> p a", p=128))
        nc.sync.dma_start(out=b2_sb, in_=b2_d.rearrange("(a p) -> p a", p=128))
        for k in range(2):
            nc.sync.dma_start(out=w2_sb[k], in_=w2_d[k])
            nc.sync.dma_start(out=w3_sb[k], in_=w3_d[k])
        for gq in range(8):
            sl = slice(gq * 8 * H, (gq + 1) * 8 * H)
            for k in range(2):
                nc.scalar.dma_start(out=g_sb[k][:, sl], in_=g_d[k][:, sl])

        # ---- forward: d1[k,b], d2[m,b] for all 1024 b ----
        h1 = [const.tile([128, BS], BF, name=f"h1_{k}") for k in range(2)]
        d1 = [const.tile([128, BS], BF, name=f"d1_{k}") for k in range(2)]
        d2 = [const.tile([128, BS], BF, name=f"d2_{m}") for m in range(2)]
        for hh in range(2):
            a_ps = ps.tile([128, BS], F32, tag="v", bufs=3, name="a1_ps")
            for s in range(BS // 512):
                nc.tensor.matmul(
                    a_ps[:, s * 512 : (s + 1) * 512],
                    w1_sb[:, hh * 128 : (hh + 1) * 128],
                    xT_sb[:, s * 512 : (s + 1) * 512],
                    start=True,
                    stop=True,
                )
            nc.scalar.activation(
                out=h1[hh], in_=a_ps, func=Tanh, bias=b1_sb[:, hh : hh + 1]
            )
            sq = sb.tile([128, BS], BF, tag="sq", name="sq1")
            nc.vector.tensor_tensor(out=sq, in0=h1[hh], in1=h1[hh], op=MUL)
            nc.vector.tensor_scalar(
                out=d1[hh], in0=sq, scalar1=-1.0, scalar2=1.0, op0=MUL, op1=ADD
            )
        for mh in range(2):
            a_ps = ps.tile([128, BS], F32, tag="v", bufs=3, name="a2_ps")
            for s in range(BS // 512):
                for hh in range(2):
                    nc.tensor.matmul(
                        a_ps[:, s * 512 : (s + 1) * 512],
                        w2_sb[hh][:, mh * 128 : (mh + 1) * 128],
                        h1[hh][:, s * 512 : (s + 1) * 512],
                        start=(hh == 0),
                        stop=(hh == 1),
                    )
            h2 = sb.tile([128, BS], BF, tag="h2", name="h2")
            nc.scalar.activation(
                out=h2, in_=a_ps, func=Tanh, bias=b2_sb[:, mh : mh + 1]
            )
            sq = sb.tile([128, BS], BF, tag="sq", name="sq2")
            nc.vector.tensor_tensor(out=sq, in0=h2, in1=h2, op=MUL)
            nc.vector.tensor_scalar(
                out=d2[mh], in0=sq, scalar1=-1.0, scalar2=1.0, op0=MUL, op1=ADD
            )

        # single w buffer per mh, WINDOW-major I-major: w[p, t*512 + i*8 + bl]
        # (t = 8-batch window). Stage-2 moving is a contiguous 512-col slice;
        # stage-1 drain writes land in contiguous 16B runs (8 bl x bf16).
        w_sb = [const.tile([128, CHUNK * D], BF, name=f"w_{m}") for m in range(2)]

        tidx = 0
        for c in range(NCH):
            cb = c * CHUNK
            # ---- stage 1: V[m,(i,b)] = sum_k G_i[k,m] d1[k,b]; w = V*d2 ----
            for q in range(NQ):
                for mh in range(2):
                    v_ps = ps.tile([128, 4 * CHUNK], F32, tag="v", bufs=3, name="v_ps")
                    for qi in range(4):
                        i = q * 4 + qi
                        goff = i * H + mh * 128
                        for kh in range(2):
                            nc.tensor.matmul(
                                v_ps[:, qi * CHUNK : (qi + 1) * CHUNK],
                                g_sb[kh][:, goff : goff + 128],
                                d1[kh][:, cb : cb + CHUNK],
                                start=(kh == 0),
                                stop=(kh == 1),
                            )
                    # drain: w[p, t*512 + i*8 + bl] = V * d2
                    # out iterates (qi, t, bl): innermost bl is 8-elem
                    # contiguous (one 16B line) - near-contiguous writes
                    wout = (
                        w_sb[mh]
                        .rearrange("p (t i bl) -> p i t bl", t=32, i=D)
                        [:, q * 4 : q * 4 + 4]
                    )
                    d2b = (
                        d2[mh][:, None, cb : cb + CHUNK]
                        .broadcast_to([128, 4, CHUNK])
                        .rearrange("p i (t bl) -> p i t bl", t=32)
                    )
                    vv = v_ps.rearrange("p (i t bl) -> p i t bl", i=4, t=32)
                    # every 3rd tile takes the ACT-copy + GPSIMD-multiply path
                    # so DVE keeps pace with the PE tile rate; keep the GP
                    # chain (copy+mult latency) out of the chunk tail
                    tl = q * 2 + mh
                    if tl % 3 == 0 and tl < 28:
                        # ACT copies psum out; GPSIMD does the multiply
                        vtmp = sb.tile([128, 4 * CHUNK], BF, tag="vtmp", bufs=3,
                                       name="vtmp")
                        nc.scalar.activation(out=vtmp, in_=v_ps, func=Copy)
                        nc.gpsimd.tensor_tensor(
                            out=wout,
                            in0=vtmp.rearrange("p (i t bl) -> p i t bl", i=4, t=32),
                            in1=d2b,
                            op=MUL,
                        )
                    else:
                        nc.vector.tensor_tensor(out=wout, in0=vv, in1=d2b, op=MUL)
                    tidx += 1

            # ---- stage 2: J[j,(b,i)]; two 64-row halves packed on partitions ----
            for t in range(CHUNK // WIN):
                j_ps = ps.tile([128, 8 * D], F32, tag="js", bufs=2, name="j_ps")
                for half in range(2):
                    bo = t * WIN + half * 8
                    for mh in range(2):
                        nc.tensor.matmul(
                            j_ps[half * 64 : (half + 1) * 64, :],
                            w3_sb[mh],
                            w_sb[mh][:, bo * D : (bo + 8) * D],
                            start=(mh == 0),
                            stop=(mh == 1),
                        )
                # psum cols are (i*8+bl) i-major; permute to (bl*64+i) for the
                # DMA here. Iterate (bl, i): strided psum READ, contiguous
                # jbuf WRITE (strided writes are ~4x slow; reads are cheap)
                jbuf = sb.tile([128, 8 * D], F32, tag="jbuf", bufs=3, name="jbuf")
                nc.scalar.activation(
                    out=jbuf,
                    in_=j_ps.rearrange("p (i bl) -> p bl i", i=D),
                    func=Copy,
                )
                b0 = cb + t * WIN
                for half in range(2):
                    nc.sync.dma_start(
                        out=jac_d[b0 + half * 8 : b0 + half * 8 + 8].rearrange(
                            "bl j i -> j bl i"
                        ),
                        in_=jbuf[half * 64 : (half + 1) * 64, :].rearrange(
                            "p (bl i) -> p bl i", bl=8
                        ),
                    )
    nc.compile()
    return nc


def kernel(x, W1, b1, W2, b2, W3, b3):
    x = np.asarray(x, dtype=np.float32)
    bf = ml_dtypes.bfloat16
    if "nc" not in _CACHE:
        _CACHE["nc"] = _build()
    nc = _CACHE["nc"]

    W1f = np.asarray(W1, np.float32)
    W2f = np.asarray(W2, np.float32)
    W3f = np.asarray(W3, np.float32)
    # G[kh][k', i*256 + m] = W1[i, kh*128+k'] * W2[kh*128+k', m]
    w1t = np.ascontiguousarray(W1f.T).reshape(2, 128, D)  # (kh, k', i)
    w2r = W2f.reshape(2, 128, H)  # (kh, k', m)
    g = (w1t[:, :, :, None] * w2r[:, :, None, :]).reshape(2, 128, D * H)

    shared = {
        "g": g.astype(bf),
        "w1": W1f.astype(bf),
        "w2": w2r.astype(bf),
        "w3": np.ascontiguousarray(W3f.reshape(2, 128, D)).astype(bf),
        "b1": np.asarray(b1, np.float32),
        "b2": np.asarray(b2, np.float32),
    }
    in_maps = [
        {
            "xt": np.ascontiguousarray(x[c * BS : (c + 1) * BS].T).astype(bf),
            **shared,
        }
        for c in range(NCORES)
    ]
    res = run_bass_kernel_spmd(
        nc, in_maps, core_ids=list(range(NCORES)), trace=TRACE
    )
    _CACHE["last_res"] = res
    return np.concatenate([r["jac"] for r in res.results], axis=0)


# revision 21
# speedup vs baseline: 1.2683x; 1.0064x over previous
"""Batch-Jacobian of a 3-layer tanh MLP (64->256->256->64), B=8192.

J[b] = W3^T diag(1-h2^2) W2^T diag(1-h1^2) W1^T   (shape 64x64 per b)

v2 strategy (per core, 1024 batch elems, 4 chunks of 256):
  host precomputes G[i][k,m] = W1[i,k]*W2[k,m]  (bf16, DMA'd in)
  forward (feature layout) -> d1[k,b], d2[m,b]
  stage1 (PE): V[m,(i,b)] = sum_k G_i[k,m] d1[k,b]   (moving = d1, no DVE build)
  drain: w = V * d2  split DVE-direct / ACT-copy + DVE-mult(2x bf16)
  stage2 (PE): J[j,(b,i)] = sum_m W3[m,j] w[m,i,b], two 64-row outputs
               packed into one 128-partition psum tile (base_partition 0/64)
  output: DMA psum -> DRAM directly (256B contiguous runs), no drain
"""

import sys

sys.path.insert(0, "/opt/trn_rl_repo")

import numpy as np
import ml_dtypes
from contextlib import ExitStack

import concourse.bass as bass
import concourse.mybir as mybir
import concourse.tile as tile
from concourse import bacc
from concourse.bass_utils import run_bass_kernel_spmd

B, D, H = 8192, 64, 256
NCORES = 8
BS = B // NCORES  # 1024 batch per core
CHUNK = 256  # jacobian batch chunk
NCH = BS // CHUNK  # 4
NQ = 16  # i-quads (4 i per quad)
WIN = 16  # batch elems per output DMA (2 halves of 8)

BF = mybir.dt.bfloat16
F32 = mybir.dt.float32
MUL = mybir.AluOpType.mult
ADD = mybir.AluOpType.add
Tanh = mybir.ActivationFunctionType.Tanh
Copy = mybir.ActivationFunctionType.Copy

_CACHE = {}
TRACE = False
# fraction pattern for DVE-direct drains: 1 of every 5 tiles
DVE_DIRECT_EVERY = 5


def _build():
    nc = bacc.Bacc("TRN2")
    xT_d = nc.dram_tensor("xt", [D, BS], BF, kind="ExternalInput")
    g_d = nc.dram_tensor("g", [2, 128, D * H], BF, kind="ExternalInput")
    w1_d = nc.dram_tensor("w1", [D, H], BF, kind="ExternalInput")
    w2_d = nc.dram_tensor("w2", [2, 128, H], BF, kind="ExternalInput")
    w3_d = nc.dram_tensor("w3", [2, 128, D], BF, kind="ExternalInput")
    b1_d = nc.dram_tensor("b1", [H], F32, kind="ExternalInput")
    b2_d = nc.dram_tensor("b2", [H], F32, kind="ExternalInput")
    jac_d = nc.dram_tensor("jac", [BS, D, D], F32, kind="ExternalOutput")

    with ExitStack() as ctx:
        tc = ctx.enter_context(tile.TileContext(nc))
        const = ctx.enter_context(tc.tile_pool(name="const", bufs=1))
        sb = ctx.enter_context(tc.tile_pool(name="sb", bufs=2))
        ps = ctx.enter_context(tc.tile_pool(name="ps", bufs=1, space="PSUM"))

        # ---- constants: small tensors first (forward can start right away);
        # G streams on the scalar HWDGE queue, kh-interleaved so stage-1's
        # first i-groups are ready early ----
        xT_sb = const.tile([D, BS], BF)
        nc.sync.dma_start(out=xT_sb, in_=xT_d[:, :])
        g_sb = [const.tile([128, D * H], BF, name=f"g{k}") for k in range(2)]
        w2_sb = [const.tile([128, H], BF, name=f"w2{k}") for k in range(2)]
        w3_sb = [const.tile([128, D], BF, name=f"w3{k}") for k in range(2)]
        w1_sb = const.tile([D, H], BF)
        nc.sync.dma_start(out=w1_sb, in_=w1_d[:, :])
        b1_sb = const.tile([128, 2], F32)
        b2_sb = const.tile([128, 2], F32)
        nc.sync.dma_start(out=b1_sb, in_=b1_d.rearrange("(a p) -> p a", p=128))
        nc.sync.dma_start(out=b2_sb, in_=b2_d.rearrange("(a p) -> p a", p=128))
        for k in range(2):
            nc.sync.dma_start(out=w2_sb[k], in_=w2_d[k])
            nc.sync.dma_start(out=w3_sb[k], in_=w3_d[k])

        # ---- forward: d1[k,b], d2[m,b] for all 1024 b ----
        h1 = [const.tile([128, BS], BF, name=f"h1_{k}") for k in range(2)]
        d1 = [const.tile([128, BS], BF, name=f"d1_{k}") for k in range(2)]
        d2 = [const.tile([128, BS], BF, name=f"d2_{m}") for m in range(2)]
        for hh in range(2):
            a_ps = ps.tile([128, BS], F32, tag="v", bufs=3, name="a1_ps")
            for s in range(BS // 512):
                nc.tensor.matmul(
                    a_ps[:, s * 512 : (s + 1) * 512],
                    w1_sb[:, hh * 128 : (hh + 1) * 128],
                    xT_sb[:, s * 512 : (s + 1) * 512],
                    start=True,
                    stop=True,
                )
            nc.scalar.activation(
                out=h1[hh], in_=a_ps, func=Tanh, bias=b1_sb[:, hh : hh + 1]
            )
            sq = sb.tile([128, BS], BF, tag="sq", name="sq1")
            nc.vector.tensor_tensor(out=sq, in0=h1[hh], in1=h1[hh], op=MUL)
            nc.vector.tensor_scalar(
                out=d1[hh], in0=sq, scalar1=-1.0, scalar2=1.0, op0=MUL, op1=ADD
            )
        for mh in range(2):
            a_ps = ps.tile([128, BS], F32, tag="v", bufs=3, name="a2_ps")
            for s in range(BS // 512):
                for hh in range(2):
                    nc.tensor.matmul(
                        a_ps[:, s * 512 : (s + 1) * 512],
                        w2_sb[hh][:, mh * 128 : (mh + 1) * 128],
                        h1[hh][:, s * 512 : (s + 1) * 512],
                        start=(hh == 0),
                        stop=(hh == 1),
                    )
            h2 = sb.tile([128, BS], BF, tag="h2", name="h2")
            nc.scalar.activation(
                out=h2, in_=a_ps, func=Tanh, bias=b2_sb[:, mh : mh + 1]
            )
            sq = sb.tile([128, BS], BF, tag="sq", name="sq2")
            nc.vector.tensor_tensor(out=sq, in0=h2, in1=h2, op=MUL)
            nc.vector.tensor_scalar(
                out=d2[mh], in0=sq, scalar1=-1.0, scalar2=1.0, op0=MUL, op1=ADD
            )

        # G loads issued AFTER the forward's engine ops (so the DMA-issue
        # instructions don't block fwd's ACT work); kh-interleaved, split
        # across both HWDGE queues. Stage-1 consumes pieces in order.
        for gq in range(8):
            sl = slice(gq * 8 * H, (gq + 1) * 8 * H)
            for k in range(2):
                eng = nc.sync if gq % 2 == 0 else nc.scalar
                eng.dma_start(out=g_sb[k][:, sl], in_=g_d[k][:, sl])

        # single w buffer per mh, WINDOW-major I-major: w[p, t*512 + i*8 + bl]
        # (t = 8-batch window). Stage-2 moving is a contiguous 512-col slice;
        # stage-1 drain writes land in contiguous 16B runs (8 bl x bf16).
        w_sb = [const.tile([128, CHUNK * D], BF, name=f"w_{m}") for m in range(2)]

        tidx = 0
        for c in range(NCH):
            cb = c * CHUNK
            # ---- stage 1: V[m,(i,b)] = sum_k G_i[k,m] d1[k,b]; w = V*d2 ----
            for q in range(NQ):
                for mh in range(2):
                    v_ps = ps.tile([128, 4 * CHUNK], F32, tag="v", bufs=3, name="v_ps")
                    for qi in range(4):
                        i = q * 4 + qi
                        goff = i * H + mh * 128
                        for kh in range(2):
                            nc.tensor.matmul(
                                v_ps[:, qi * CHUNK : (qi + 1) * CHUNK],
                                g_sb[kh][:, goff : goff + 128],
                                d1[kh][:, cb : cb + CHUNK],
                                start=(kh == 0),
                                stop=(kh == 1),
                            )
                    # drain: w[p, t*512 + i*8 + bl] = V * d2
                    # out iterates (qi, t, bl): innermost bl is 8-elem
                    # contiguous (one 16B line) - near-contiguous writes
                    wout = (
                        w_sb[mh]
                        .rearrange("p (t i bl) -> p i t bl", t=32, i=D)
                        [:, q * 4 : q * 4 + 4]
                    )
                    d2b = (
                        d2[mh][:, None, cb : cb + CHUNK]
                        .broadcast_to([128, 4, CHUNK])
                        .rearrange("p i (t bl) -> p i t bl", t=32)
                    )
                    vv = v_ps.rearrange("p (i t bl) -> p i t bl", i=4, t=32)
                    # drain split tuned to engine rates (DVE-direct 1197ns,
                    # ACT-copy 953, GP-mult 2125, DVE-mult ~690 per tile):
                    # 16 DVE-direct / 11 ACT+GP / 5 ACT+DVE-mult per chunk,
                    # long-latency GP chains kept out of the chunk tail
                    tl = q * 2 + mh
                    copy_tile = (tl % 2 == 1 and tl < 28) or tl in (24, 26)
                    if copy_tile:
                        vtmp = sb.tile([128, 4 * CHUNK], BF, tag="vtmp", bufs=3,
                                       name="vtmp")
                        nc.scalar.activation(out=vtmp, in_=v_ps, func=Copy)
                        meng = nc.vector if tl % 3 == 0 else nc.gpsimd
                        meng.tensor_tensor(
                            out=wout,
                            in0=vtmp.rearrange("p (i t bl) -> p i t bl", i=4, t=32),
                            in1=d2b,
                            op=MUL,
                        )
                    else:
                        nc.vector.tensor_tensor(out=wout, in0=vv, in1=d2b, op=MUL)
                    tidx += 1

            # ---- stage 2: J[j,(b,i)]; two 64-row halves packed on partitions ----
            for t in range(CHUNK // WIN):
                j_ps = ps.tile([128, 8 * D], F32, tag="js", bufs=2, name="j_ps")
                for half in range(2):
                    bo = t * WIN + half * 8
                    for mh in range(2):
                        nc.tensor.matmul(
                            j_ps[half * 64 : (half + 1) * 64, :],
                            w3_sb[mh],
                            w_sb[mh][:, bo * D : (bo + 8) * D],
                            start=(mh == 0),
                            stop=(mh == 1),
                        )
                # psum cols are (i*8+bl) i-major; permute to (bl*64+i) for the
                # DMA here. Iterate (bl, i): strided psum READ, contiguous
                # jbuf WRITE (strided writes are ~4x slow; reads are cheap)
                jbuf = sb.tile([128, 8 * D], F32, tag="jbuf", bufs=3, name="jbuf")
                nc.scalar.activation(
                    out=jbuf,
                    in_=j_ps.rearrange("p (i bl) -> p bl i", i=D),
                    func=Copy,
                )
                b0 = cb + t * WIN
                for half in range(2):
                    nc.sync.dma_start(
                        out=jac_d[b0 + half * 8 : b0 + half * 8 + 8].rearrange(
                            "bl j i -> j bl i"
                        ),
                        in_=jbuf[half * 64 : (half + 1) * 64, :].rearrange(
                            "p (bl i) -> p bl i", bl=8
                        ),
                    )
    nc.compile()
    return nc


def kernel(x, W1, b1, W2, b2, W3, b3):
    x = np.asarray(x, dtype=np.float32)
    bf = ml_dtypes.bfloat16
    if "nc" not in _CACHE:
        _CACHE["nc"] = _build()
    nc = _CACHE["nc"]

    W1f = np.asarray(W1, np.float32)
    W2f = np.asarray(W2, np.float32)
    W3f = np.asarray(W3, np.float32)
    # G[kh][k', i*256 + m] = W1[i, kh*128+k'] * W2[kh*128+k', m]
    w1t = np.ascontiguousarray(W1f.T).reshape(2, 128, D)  # (kh, k', i)
    w2r = W2f.reshape(2, 128, H)  # (kh, k', m)
    g = (w1t[:, :, :, None] * w2r[:, :, None, :]).reshape(2, 128, D * H)

    shared = {
        "g": g.astype(bf),
        "w1": W1f.astype(bf),
        "w2": w2r.astype(bf),
        "w3": np.ascontiguousarray(W3f.reshape(2, 128, D)).astype(bf),
        "b1": np.asarray(b1, np.float32),
        "b2": np.asarray(b2, np.float32),
    }
    in_maps = [
        {
            "xt": np.ascontiguousarray(x[c * BS : (c + 1) * BS].T).astype(bf),
            **shared,
        }
        for c in range(NCORES)
    ]
    res = run_bass_kernel_spmd(
        nc, in_maps, core_ids=list(range(NCORES)), trace=TRACE
    )
    _CACHE["last_res"] = res
    return np.concatenate([r["jac"] for r in res.results], axis=0)


# revision 22
# speedup vs baseline: 1.3455x; 1.0609x over previous
"""Batch-Jacobian of a 3-layer tanh MLP (64->256->256->64), B=8192.

J[b] = W3^T diag(1-h2^2) W2^T diag(1-h1^2) W1^T   (shape 64x64 per b)

v2 strategy (per core, 1024 batch elems, 4 chunks of 256):
  host precomputes G[i][k,m] = W1[i,k]*W2[k,m]  (bf16, DMA'd in)
  forward (feature layout) -> d1[k,b], d2[m,b]
  stage1 (PE): V[m,(i,b)] = sum_k G_i[k,m] d1[k,b]   (moving = d1, no DVE build)
  drain: w = V * d2  split DVE-direct / ACT-copy + DVE-mult(2x bf16)
  stage2 (PE): J[j,(b,i)] = sum_m W3[m,j] w[m,i,b], two 64-row outputs
               packed into one 128-partition psum tile (base_partition 0/64)
  output: DMA psum -> DRAM directly (256B contiguous runs), no drain
"""

import sys

sys.path.insert(0, "/opt/trn_rl_repo")

import numpy as np
import ml_dtypes
from contextlib import ExitStack

import concourse.bass as bass
import concourse.mybir as mybir
import concourse.tile as tile
from concourse import bacc
from concourse.bass_utils import run_bass_kernel_spmd

B, D, H = 8192, 64, 256
NCORES = 8
BS = B // NCORES  # 1024 batch per core
CHUNK = 256  # jacobian batch chunk
NCH = BS // CHUNK  # 4
NQ = 16  # i-quads (4 i per quad)
WIN = 16  # batch elems per output DMA (2 halves of 8)

BF = mybir.dt.bfloat16
F32 = mybir.dt.float32
MUL = mybir.AluOpType.mult
ADD = mybir.AluOpType.add
Tanh = mybir.ActivationFunctionType.Tanh
Copy = mybir.ActivationFunctionType.Copy

_CACHE = {}
TRACE = False
# fraction pattern for DVE-direct drains: 1 of every 5 tiles
DVE_DIRECT_EVERY = 5


def _build():
    nc = bacc.Bacc("TRN2")
    xT_d = nc.dram_tensor("xt", [D, BS], BF, kind="ExternalInput")
    g_d = nc.dram_tensor("g", [2, 128, D * H], BF, kind="ExternalInput")
    w1_d = nc.dram_tensor("w1", [D, H], BF, kind="ExternalInput")
    w2_d = nc.dram_tensor("w2", [2, 128, H], BF, kind="ExternalInput")
    w3_d = nc.dram_tensor("w3", [2, 128, D], BF, kind="ExternalInput")
    b1_d = nc.dram_tensor("b1", [H], F32, kind="ExternalInput")
    b2_d = nc.dram_tensor("b2", [H], F32, kind="ExternalInput")
    jac_d = nc.dram_tensor("jac", [BS, D, D], F32, kind="ExternalOutput")

    with ExitStack() as ctx:
        tc = ctx.enter_context(tile.TileContext(nc))
        const = ctx.enter_context(tc.tile_pool(name="const", bufs=1))
        sb = ctx.enter_context(tc.tile_pool(name="sb", bufs=2))
        ps = ctx.enter_context(tc.tile_pool(name="ps", bufs=1, space="PSUM"))

        # ---- constants: small tensors first (forward can start right away);
        # G streams on the scalar HWDGE queue, kh-interleaved so stage-1's
        # first i-groups are ready early ----
        xT_sb = const.tile([D, BS], BF)
        nc.sync.dma_start(out=xT_sb, in_=xT_d[:, :])
        g_sb = [const.tile([128, D * H], BF, name=f"g{k}") for k in range(2)]
        w2_sb = [const.tile([128, H], BF, name=f"w2{k}") for k in range(2)]
        w3_sb = [const.tile([128, D], BF, name=f"w3{k}") for k in range(2)]
        w1_sb = const.tile([D, H], BF)
        nc.sync.dma_start(out=w1_sb, in_=w1_d[:, :])
        b1_sb = const.tile([128, 2], F32)
        b2_sb = const.tile([128, 2], F32)
        nc.sync.dma_start(out=b1_sb, in_=b1_d.rearrange("(a p) -> p a", p=128))
        nc.sync.dma_start(out=b2_sb, in_=b2_d.rearrange("(a p) -> p a", p=128))
        for k in range(2):
            nc.sync.dma_start(out=w2_sb[k], in_=w2_d[k])
            nc.sync.dma_start(out=w3_sb[k], in_=w3_d[k])

        # ---- forward: d1[k,b], d2[m,b] for all 1024 b ----
        h1 = [const.tile([128, BS], BF, name=f"h1_{k}") for k in range(2)]
        d1 = [const.tile([128, BS], BF, name=f"d1_{k}") for k in range(2)]
        d2 = [const.tile([128, BS], BF, name=f"d2_{m}") for m in range(2)]
        for hh in range(2):
            a_ps = ps.tile([128, BS], F32, tag="v", bufs=3, name="a1_ps")
            for s in range(BS // 512):
                nc.tensor.matmul(
                    a_ps[:, s * 512 : (s + 1) * 512],
                    w1_sb[:, hh * 128 : (hh + 1) * 128],
                    xT_sb[:, s * 512 : (s + 1) * 512],
                    start=True,
                    stop=True,
                )
            nc.scalar.activation(
                out=h1[hh], in_=a_ps, func=Tanh, bias=b1_sb[:, hh : hh + 1]
            )
            sq = sb.tile([128, BS], BF, tag="sq", name="sq1")
            nc.vector.tensor_tensor(out=sq, in0=h1[hh], in1=h1[hh], op=MUL)
            nc.vector.tensor_scalar(
                out=d1[hh], in0=sq, scalar1=-1.0, scalar2=1.0, op0=MUL, op1=ADD
            )
        for mh in range(2):
            a_ps = ps.tile([128, BS], F32, tag="v", bufs=3, name="a2_ps")
            for s in range(BS // 512):
                for hh in range(2):
                    nc.tensor.matmul(
                        a_ps[:, s * 512 : (s + 1) * 512],
                        w2_sb[hh][:, mh * 128 : (mh + 1) * 128],
                        h1[hh][:, s * 512 : (s + 1) * 512],
                        start=(hh == 0),
                        stop=(hh == 1),
                    )
            h2 = sb.tile([128, BS], BF, tag="h2", name="h2")
            nc.scalar.activation(
                out=h2, in_=a_ps, func=Tanh, bias=b2_sb[:, mh : mh + 1]
            )
            sq = sb.tile([128, BS], BF, tag="sq", name="sq2")
            nc.vector.tensor_tensor(out=sq, in0=h2, in1=h2, op=MUL)
            nc.vector.tensor_scalar(
                out=d2[mh], in0=sq, scalar1=-1.0, scalar2=1.0, op0=MUL, op1=ADD
            )

        # G loads issued AFTER the forward's engine ops (so the DMA-issue
        # instructions don't block fwd's ACT work); kh-interleaved, split
        # across both HWDGE queues. Stage-1 consumes pieces in order.
        for gq in range(8):
            sl = slice(gq * 8 * H, (gq + 1) * 8 * H)
            for k in range(2):
                eng = nc.sync if gq % 2 == 0 else nc.scalar
                eng.dma_start(out=g_sb[k][:, sl], in_=g_d[k][:, sl])

        # single w buffer per mh, WINDOW-major I-major: w[p, t*512 + i*8 + bl]
        # (t = 8-batch window). Stage-2 moving is a contiguous 512-col slice;
        # stage-1 drain writes land in contiguous 16B runs (8 bl x bf16).
        w_sb = [const.tile([128, CHUNK * D], BF, name=f"w_{m}") for m in range(2)]

        tidx = 0
        for c in range(NCH):
            cb = c * CHUNK
            # ---- stage 1: V[m,(i,b)] = sum_k G_i[k,m] d1[k,b]; w = V*d2 ----
            for q in range(NQ):
                for mh in range(2):
                    v_ps = ps.tile([128, 4 * CHUNK], F32, tag="v", bufs=3, name="v_ps")
                    for qi in range(4):
                        i = q * 4 + qi
                        goff = i * H + mh * 128
                        for kh in range(2):
                            nc.tensor.matmul(
                                v_ps[:, qi * CHUNK : (qi + 1) * CHUNK],
                                g_sb[kh][:, goff : goff + 128],
                                d1[kh][:, cb : cb + CHUNK],
                                start=(kh == 0),
                                stop=(kh == 1),
                            )
                    # drain: w[p, t*512 + i*8 + bl] = V * d2
                    # out iterates (qi, t, bl): innermost bl is 8-elem
                    # contiguous (one 16B line) - near-contiguous writes
                    wout = (
                        w_sb[mh]
                        .rearrange("p (t i bl) -> p i t bl", t=32, i=D)
                        [:, q * 4 : q * 4 + 4]
                    )
                    d2b = (
                        d2[mh][:, None, cb : cb + CHUNK]
                        .broadcast_to([128, 4, CHUNK])
                        .rearrange("p i (t bl) -> p i t bl", t=32)
                    )
                    vv = v_ps.rearrange("p (i t bl) -> p i t bl", i=4, t=32)
                    # drain split tuned to engine rates (DVE-direct 1197ns,
                    # ACT-copy 953, GP-mult 2125, DVE-mult ~690 per tile):
                    # 16 DVE-direct / 11 ACT+GP / 5 ACT+DVE-mult per chunk,
                    # long-latency GP chains kept out of the chunk tail
                    tl = q * 2 + mh
                    copy_tile = (tl % 2 == 1 and tl < 28) or tl in (24, 26)
                    if copy_tile:
                        vtmp = sb.tile([128, 4 * CHUNK], BF, tag="vtmp", bufs=3,
                                       name="vtmp")
                        nc.scalar.activation(out=vtmp, in_=v_ps, func=Copy)
                        meng = nc.vector if tl % 3 == 0 else nc.gpsimd
                        meng.tensor_tensor(
                            out=wout,
                            in0=vtmp.rearrange("p (i t bl) -> p i t bl", i=4, t=32),
                            in1=d2b,
                            op=MUL,
                        )
                    else:
                        nc.vector.tensor_tensor(out=wout, in0=vv, in1=d2b, op=MUL)
                    tidx += 1

            # ---- stage 2: J[j,(b,i)]; two 64-row halves packed on partitions ----
            for t in range(CHUNK // WIN):
                j_ps = ps.tile([128, 8 * D], F32, tag="js", bufs=2, name="j_ps")
                for half in range(2):
                    bo = t * WIN + half * 8
                    for mh in range(2):
                        nc.tensor.matmul(
                            j_ps[half * 64 : (half + 1) * 64, :],
                            w3_sb[mh],
                            w_sb[mh][:, bo * D : (bo + 8) * D],
                            start=(mh == 0),
                            stop=(mh == 1),
                        )
                # psum cols are (i*8+bl) i-major; permute to (bl*64+i) for the
                # DMA here. Iterate (bl, i): strided psum READ, contiguous
                # jbuf WRITE (strided writes are ~4x slow; reads are cheap).
                # Copies alternate DVE/ACT; DMA issues alternate sync/scalar
                # queues; deep jbuf ring rides out DMA completion latency.
                jbuf = sb.tile([128, 8 * D], F32, tag="jbuf", bufs=6, name="jbuf")
                jsrc = j_ps.rearrange("p (i bl) -> p bl i", i=D)
                if t % 2 == 0:
                    nc.scalar.activation(out=jbuf, in_=jsrc, func=Copy)
                else:
                    nc.vector.tensor_scalar(
                        out=jbuf, in0=jsrc, scalar1=1.0, scalar2=0.0,
                        op0=MUL, op1=ADD,
                    )
                b0 = cb + t * WIN
                for half in range(2):
                    deng = nc.sync if (t * 2 + half) % 2 == 0 else nc.scalar
                    deng.dma_start(
                        out=jac_d[b0 + half * 8 : b0 + half * 8 + 8].rearrange(
                            "bl j i -> j bl i"
                        ),
                        in_=jbuf[half * 64 : (half + 1) * 64, :].rearrange(
                            "p (bl i) -> p bl i", bl=8
                        ),
                    )
    nc.compile()
    return nc


def kernel(x, W1, b1, W2, b2, W3, b3):
    x = np.asarray(x, dtype=np.float32)
    bf = ml_dtypes.bfloat16
    if "nc" not in _CACHE:
        _CACHE["nc"] = _build()
    nc = _CACHE["nc"]

    W1f = np.asarray(W1, np.float32)
    W2f = np.asarray(W2, np.float32)
    W3f = np.asarray(W3, np.float32)
    # G[kh][k', i*256 + m] = W1[i, kh*128+k'] * W2[kh*128+k', m]
    w1t = np.ascontiguousarray(W1f.T).reshape(2, 128, D)  # (kh, k', i)
    w2r = W2f.reshape(2, 128, H)  # (kh, k', m)
    g = (w1t[:, :, :, None] * w2r[:, :, None, :]).reshape(2, 128, D * H)

    shared = {
        "g": g.astype(bf),
        "w1": W1f.astype(bf),
        "w2": w2r.astype(bf),
        "w3": np.ascontiguousarray(W3f.reshape(2, 128, D)).astype(bf),
        "b1": np.asarray(b1, np.float32),
        "b2": np.asarray(b2, np.float32),
    }
    in_maps = [
        {
            "xt": np.ascontiguousarray(x[c * BS : (c + 1) * BS].T).astype(bf),
            **shared,
        }
        for c in range(NCORES)
    ]
    res = run_bass_kernel_spmd(
        nc, in_maps, core_ids=list(range(NCORES)), trace=TRACE
    )
    _CACHE["last_res"] = res
    return np.concatenate([r["jac"] for r in res.results], axis=0)
